# revision 6
# baseline (speedup 1.0000x reference)
"""Trainium2 Bass kernel for nn_DNM_Conv (LayerNorm -> synapse contraction ->
dendritic weighting -> GELU -> residual multiply).

Algebraic reduction of the reference:
    y = LayerNorm(x)                                  (b, n, d)
    t[b,o,d] = sum_n W[o,n] * y[b,n,d] + c[o]
        where W[o,n] = sum_m dw[o,m]*sw[o,m,n],  c[o] = sum_{m,n} dw[o,m]*sb[o,m,n]
    out = x * (gelu_erf(t) + 1)                       (o == n == 196)

Kernel structure (v3):
  * All inputs shipped partition-major so every load is one DMA with one
    descriptor per partition (the DMA trigger instruction cost scales with
    descriptor count).
  * LN stats via PE: host ships [x^T | (x^2)^T] in fp8e4; matmuls with
    basis-column weights reduce over d, landing [sum x | sum x^2] per batch
    as rows of a PSUM tile. Two batch-groups of 4 pipeline stats against
    the stats post-processing. Dummy matmuls warm the PE (HAM) during the
    input DMA.
  * mean/var/rstd computed on the [4,196] row layout (DVE+ACT), then
    PE-transposed to per-partition columns.
  * Normalization applied to the rhs: y' = rstd*x - rstd*mu via one fused
    DVE tensor_scalar per (batch, n-chunk). Matmul weights are the constant
    W^T for all batches and the GELU bias is just c.
  * GELU reads both d-chunk PSUM banks in one instruction (FD=768).
  * Output: (g+1) in-place at DVE 4x then one tensor_tensor multiply by x
    per (pair, o-chunk); single DRAM write, no residual seed/accumulate.
  * ACT table sets: dummy activations preload the sqrt and gelu tables
    off the critical path.

Distribution: data-parallel over batch, 8 batches per core on 8 cores.
Datapath fp16 (host casts), fp8 stats operands, fp32 PSUM + LN statistics.
"""

import numpy as np

B, N, D, O, M = 64, 196, 768, 196, 2
N_CORES = 8
BPC = B // N_CORES          # batches per core
NPAIR = BPC // 2            # batch pairs
NG = 2                      # stats batch groups
GB = BPC // NG              # batches per group (4)
NA, NB = 128, 68            # n (and o) partition split
DC = 384                    # matmul free-dim chunk (one PSUM bank)
NDCH = D // 128             # d-chunks for the stats matmuls (6)
SQ = 2 * N                  # stats row width: [sum x | sum x^2]
LN_EPS = 1e-5
N_WARM = 28                 # PE warm-up dummy matmuls

_NC_CACHE = {}


def _build_nc(nontrivial_ln):
    import concourse.bacc as bacc
    import concourse.tile as tile
    import concourse.bass as bass
    from concourse.tile import add_dep_helper
    from concourse import mybir
    from contextlib import ExitStack
    import ml_dtypes

    F32 = mybir.dt.float32
    F16 = mybir.dt.float16
    F8 = mybir.dt.float8e4
    AF = mybir.ActivationFunctionType
    OP = mybir.AluOpType

    nc = bacc.Bacc()
    xa_d = nc.declare_dram_parameter("xa", [NA, NPAIR, 2, D], F16, isOutput=False)
    xb_d = nc.declare_dram_parameter("xb", [NB, NPAIR, 2, D], F16, isOutput=False)
    xs_d = nc.declare_dram_parameter("xs", [128, NG, NDCH, GB, SQ], F8,
                                     isOutput=False)
    wt_d = nc.declare_dram_parameter("wt", [N, O], F16, isOutput=False)
    c_d = nc.declare_dram_parameter("c", [O, 1], F32, isOutput=False)
    if nontrivial_ln:
        lnw_d = nc.declare_dram_parameter("lnw", [1, 2, DC], F32, isOutput=False)
        lnbe_d = nc.declare_dram_parameter("lnbe", [O, D], F32, isOutput=False)
    oa_d = nc.declare_dram_parameter("oa", [NA, NPAIR, 2, D], F16, isOutput=True)
    ob_d = nc.declare_dram_parameter("ob", [NB, NPAIR, 2, D], F16, isOutput=True)

    # basis columns for the stats matmuls: ebt[p, b, m] = 1 if m == b else 0
    eb_np = np.broadcast_to(np.eye(GB, dtype=np.float32), (128, GB, GB))
    eye_np = np.eye(GB, dtype=np.float32)

    nsplit = ((0, NA), (NA, NB))

    with tile.TileContext(nc) as tc, ExitStack() as ctx:
        const = ctx.enter_context(tc.tile_pool(name="const", bufs=1))
        xspool = ctx.enter_context(tc.tile_pool(name="xspool", bufs=1))
        xpool = ctx.enter_context(tc.tile_pool(name="xpool", bufs=1))
        ypool = ctx.enter_context(tc.tile_pool(name="ypool", bufs=2))
        gpool = ctx.enter_context(tc.tile_pool(name="gpool", bufs=2))
        opool = ctx.enter_context(tc.tile_pool(name="opool", bufs=2))
        small = ctx.enter_context(tc.tile_pool(name="small", bufs=1))
        psum = ctx.enter_context(tc.tile_pool(name="psum", bufs=2, space="PSUM"))

        # ---- input loads: big tensors first on the sync queue ----
        xst = []
        for g in range(NG):
            t = xspool.tile([128, NDCH, GB, SQ], F8, tag=f"xs{g}", name=f"xs{g}")
            nc.sync.dma_start(out=t[:], in_=xs_d[:, g])
            xst.append(t)
        xa_t = xpool.tile([NA, NPAIR, 2, D], F16, tag="xa")
        nc.sync.dma_start(out=xa_t[:], in_=xa_d.ap())
        xb_t = xpool.tile([NB, NPAIR, 2, D], F16, tag="xb")
        nc.sync.dma_start(out=xb_t[:], in_=xb_d.ap())
        xtiles = (xa_t, xb_t)

        # ---- small constants on the scalar queue ----
        ebt_h = nc.inline_tensor(eb_np.astype(ml_dtypes.float8_e4m3), name="ebt")
        eye_h = nc.inline_tensor(eye_np, name="eyeg")
        ebt = const.tile([128, GB, GB], F8, tag="ebt")
        eyeg = const.tile([GB, GB], F32, tag="eyeg")
        nc.scalar.dma_start(out=ebt[:], in_=ebt_h.ap())
        nc.scalar.dma_start(out=eyeg[:], in_=eye_h.ap())
        wt_a = const.tile([NA, O], F16, tag="wt_a")
        wt_b = const.tile([NB, O], F16, tag="wt_b")
        nc.scalar.dma_start(out=wt_a[:], in_=wt_d[0:NA, :])
        nc.scalar.dma_start(out=wt_b[:], in_=wt_d[NA:N, :])
        c_a = const.tile([NA, 1], F32, tag="c_a")
        c_b = const.tile([NB, 1], F32, tag="c_b")
        nc.scalar.dma_start(out=c_a[:], in_=c_d[0:NA, :])
        nc.scalar.dma_start(out=c_b[:], in_=c_d[NA:O, :])
        if nontrivial_ln:
            lnw_t = const.tile([128, 2, DC], F32, tag="lnw")
            lnw_bcast = bass.AP(tensor=lnw_d.ap().tensor, offset=0,
                                ap=[[0, 128], [DC, 2], [1, DC]])
            nc.scalar.dma_start(out=lnw_t[:], in_=lnw_bcast)
            lnbe_a = const.tile([NA, D], F32, tag="lnbe_a")
            lnbe_b = const.tile([NB, D], F32, tag="lnbe_b")
            nc.scalar.dma_start(out=lnbe_a[:], in_=lnbe_d[0:NA, :])
            nc.scalar.dma_start(out=lnbe_b[:], in_=lnbe_d[NA:O, :])

        # ---- ACT table preload #1 (sqrt set) + misc consts ----
        eps_t = const.tile([GB, 1], F32, tag="eps")
        nc.vector.memset(eps_t[:], LN_EPS)
        zero_t = const.tile([GB, 1], F32, tag="zero")
        nc.vector.memset(zero_t[:], 0.0)
        warm16 = const.tile([128, 128], F16, tag="warm16")
        nc.vector.memset(warm16[:], 0.0)
        scr = small.tile([1, 1], F32, tag="scr")
        nc.scalar.activation(out=scr[:], in_=c_a[0:1, 0:1],
                             func=AF.Abs_reciprocal_sqrt,
                             bias=eps_t[0:1, :], scale=0.0)

        # ---- PE warm-up during the input DMA (HAM) ----
        warm_ps = psum.tile([GB, SQ], F32, tag="pA", name="warm_ps")
        for w in range(N_WARM):
            nc.tensor.matmul(warm_ps[0:1, 0:128], warm16[:, 0:1],
                             warm16[:, 0:128], start=True, stop=True,
                             skip_group_check=True)

        # ---- PE stats per batch-group: one accumulation group each ----
        stats_ps = []
        for g in range(NG):
            sp = psum.tile([GB, SQ], F32, tag=("pA" if g == 0 else "pB"),
                           name=f"stats{g}")
            for ch in range(NDCH):
                for b in range(GB):
                    nc.tensor.matmul(sp[:], ebt[:, b, :], xst[g][:, ch, b, :],
                                     start=(ch == 0 and b == 0),
                                     stop=(ch == NDCH - 1 and b == GB - 1),
                                     skip_group_check=True)
            stats_ps.append(sp)

        # ---- stats post-processing per group (row layout [4, 196]) ----
        rstd_ins = None
        rows = []
        for g in range(NG):
            sp = stats_ps[g]
            mu_r = small.tile([GB, N], F32, tag=f"mu{g}", name=f"mu{g}")
            nc.vector.tensor_scalar_mul(out=mu_r[:], in0=sp[:, 0:N],
                                        scalar1=1.0 / D)
            musq_r = small.tile([GB, N], F32, tag=f"musq{g}", name=f"musq{g}")
            nc.vector.tensor_mul(out=musq_r[:], in0=mu_r[:], in1=mu_r[:])
            var_r = small.tile([GB, N], F32, tag=f"var{g}", name=f"var{g}")
            nc.vector.scalar_tensor_tensor(out=var_r[:], in0=sp[:, N:SQ],
                                           scalar=1.0 / D, in1=musq_r[:],
                                           op0=OP.mult, op1=OP.subtract)
            rstd_r = small.tile([GB, N], F32, tag=f"rstd{g}", name=f"rstd{g}")
            ins = nc.scalar.activation(out=rstd_r[:], in_=var_r[:],
                                       func=AF.Abs_reciprocal_sqrt,
                                       bias=eps_t[:], scale=1.0)
            if g == 0:
                rstd_ins = ins
            z_r = small.tile([GB, N], F32, tag=f"z{g}", name=f"z{g}")
            nc.vector.tensor_mul(out=z_r[:], in0=mu_r[:], in1=rstd_r[:])
            rows.append((rstd_r, z_r))

        # ---- ACT table preload #2 (gelu set), after the sqrt-set ops ----
        scr2 = small.tile([1, 1], F32, tag="scr2")
        g_pre = nc.scalar.activation(out=scr2[:], in_=c_a[0:1, 0:1],
                                     func=AF.Gelu, bias=zero_t[0:1, :],
                                     scale=1.0)
        add_dep_helper(g_pre.ins, rstd_ins.ins, sync=True,
                       reason="gelu table after sqrt-set rstd")

        # ---- transpose rstd/z rows -> per-partition columns ----
        cols = {}  # (name, ci, g) -> [pn, GB] f32 sbuf
        for g in range(NG):
            rstd_r, z_r = rows[g]
            for nm, row in (("rstd", rstd_r), ("z", z_r)):
                for ci, (p0, pn) in enumerate(nsplit):
                    ps_t = psum.tile([pn, GB], F32,
                                     tag=("pA" if ci == 0 else "pB"),
                                     name=f"tp_{nm}{ci}_{g}")
                    nc.tensor.transpose(ps_t[:], row[:, p0:p0 + pn], eyeg[:])
                    sb_t = small.tile([pn, GB], F32, tag=f"{nm}T{ci}{g}",
                                      name=f"{nm}T{ci}{g}")
                    nc.vector.tensor_copy(sb_t[:], ps_t[:])
                    cols[(nm, ci, g)] = sb_t

        # ---- per-batch pipeline ----
        ytiles = {}
        for bb in range(BPC):
            q, j = divmod(bb, 2)
            g, bl = divmod(bb, GB)
            # y' = rstd*x - z  (fused, per n-chunk)
            if j == 0:
                ytiles[q] = []
            for ci, (p0, pn) in enumerate(nsplit):
                if j == 0:
                    yt = ypool.tile([pn, 2, D], F16, tag=f"y{ci}",
                                    name=f"y{q}_{ci}")
                    ytiles[q].append(yt)
                yt = ytiles[q][ci]
                nc.vector.tensor_scalar(out=yt[:, j, :],
                                        in0=xtiles[ci][:, q, j, :],
                                        scalar1=cols[("rstd", ci, g)][:, bl:bl + 1],
                                        scalar2=cols[("z", ci, g)][:, bl:bl + 1],
                                        op0=OP.mult, op1=OP.subtract)

            # main matmuls: t = wt.T @ y' (+c as gelu bias)
            if j == 0:
                gts = {}
            for oc, (o0, on) in enumerate(nsplit):
                pm = psum.tile([on, 2, 512], F32, tag=("pA" if oc == 0 else "pB"),
                               name=f"pm{bb}_{oc}")
                for k, (p0, pn) in enumerate(nsplit):
                    wt_t = wt_a if k == 0 else wt_b
                    for dc in range(2):
                        nc.tensor.matmul(pm[:, dc, 0:DC],
                                         wt_t[:, o0:o0 + on],
                                         ytiles[q][k][:, j, dc * DC:(dc + 1) * DC],
                                         start=(k == 0), stop=(k == 1),
                                         skip_group_check=True)
                if nontrivial_ln:
                    lnbe_t = lnbe_a if oc == 0 else lnbe_b
                    nc.vector.tensor_mul(out=pm[:, :, 0:DC], in0=pm[:, :, 0:DC],
                                         in1=lnw_t[0:on, :, :])
                    nc.vector.tensor_add(
                        out=pm[:, :, 0:DC], in0=pm[:, :, 0:DC],
                        in1=lnbe_t[:, :].rearrange("p (a f) -> p a f", a=2))

                # gelu over both d-chunks (two PSUM banks) at once
                if j == 0:
                    gt = gpool.tile([on, 2, D], F16, tag=f"g{oc}",
                                    name=f"g{q}_{oc}")
                    gts[oc] = gt
                gt = gts[oc]
                c_t = c_a if oc == 0 else c_b
                nc.scalar.activation(
                    out=gt[:, j, :].rearrange("p (a f) -> p a f", a=2),
                    in_=pm[:, :, 0:DC], func=AF.Gelu, bias=c_t[:], scale=1.0)

            # pair complete: (g+1) in place (4x), multiply by x (2x), store
            if j == 1:
                for oc, (o0, on) in enumerate(nsplit):
                    nc.vector.tensor_scalar_add(out=gts[oc][:], in0=gts[oc][:],
                                                scalar1=1.0)
                    ot = opool.tile([on, 2, D], F16, tag=f"o{oc}",
                                    name=f"o{q}_{oc}")
                    nc.vector.tensor_mul(out=ot[:], in0=gts[oc][:],
                                         in1=xtiles[oc][:, q, :, :])
                    o_d = oa_d if oc == 0 else ob_d
                    nc.gpsimd.dma_start(out=o_d[:, q], in_=ot[:])

    nc.compile()
    return nc


def kernel(x, ln_w, ln_b, sw, sb, dw, _trace=False):
    import ml_dtypes
    from concourse.bass_utils import run_bass_kernel_spmd

    x = np.asarray(x, dtype=np.float32)
    ln_w = np.asarray(ln_w, dtype=np.float32)
    ln_b = np.asarray(ln_b, dtype=np.float32)
    sw = np.asarray(sw, dtype=np.float32)
    sb = np.asarray(sb, dtype=np.float32)
    dw = np.asarray(dw, dtype=np.float32)

    x16 = x.astype(np.float16)
    # partition-major x: [n-chunk][pair, j, d]
    xr = x16.reshape(N_CORES, NPAIR, 2, N, D)

    # stats operand: [x^T | (x^2)^T] per batch, partition-major, fp8
    xt = x.transpose(0, 2, 1)                       # (B, 768, 196)
    xs = np.concatenate([xt, xt * xt], axis=-1)     # (B, 768, 392)
    # -> [core][128(p), group, d-chunk, batch-in-group, SQ]
    xs = xs.reshape(N_CORES, NG, GB, NDCH, 128, SQ).transpose(0, 4, 1, 3, 2, 5)
    xs8 = np.ascontiguousarray(xs.astype(ml_dtypes.float8_e4m3))

    # Fold dendritic weights into the synapse contraction (host, ~0.1 ms).
    W = np.einsum("om,omn->on", dw, sw)            # (o, n)
    WT = np.ascontiguousarray(W.T.astype(np.float16))
    c = np.einsum("om,om->o", dw, sb.sum(-1)).astype(np.float32)[:, None]

    nontrivial_ln = not (np.all(ln_w == 1.0) and np.all(ln_b == 0.0))
    key = bool(nontrivial_ln)
    if key not in _NC_CACHE:
        _NC_CACHE[key] = _build_nc(nontrivial_ln)
    nc = _NC_CACHE[key]

    in_maps = []
    for i in range(N_CORES):
        xi = xr[i].transpose(2, 0, 1, 3)           # (196, NPAIR, 2, D)
        m = {"xa": np.ascontiguousarray(xi[0:NA]),
             "xb": np.ascontiguousarray(xi[NA:N]),
             "xs": xs8[i], "wt": WT, "c": c}
        if nontrivial_ln:
            m["lnw"] = ln_w.reshape(1, 2, DC)
            m["lnbe"] = (W.sum(-1)[:, None] * ln_b[None, :]).astype(np.float32)
        in_maps.append(m)

    res = run_bass_kernel_spmd(nc, in_maps, core_ids=list(range(N_CORES)),
                               trace=_trace)
    out = np.empty((B, N, D), dtype=np.float16)
    outr = out.reshape(N_CORES, NPAIR, 2, N, D)
    for i in range(N_CORES):
        oa = res.results[i]["oa"]                  # (NA, NPAIR, 2, D)
        ob = res.results[i]["ob"]
        outr[i, :, :, 0:NA] = oa.transpose(1, 2, 0, 3)
        outr[i, :, :, NA:N] = ob.transpose(1, 2, 0, 3)
    out = out.astype(np.float32)
    if _trace:
        return out, res
    return out
